# revision 35
# baseline (speedup 1.0000x reference)
"""Trainium2 Bass kernel for nn_GCNEdgeBasedEdgeGenCluster (3-layer edge-GCN).

Strategy (8 NeuronCores, src-owner sharding):
  - Nodes are relabeled by (degree-sorted, dealt round-robin) into 32 "units"
    = 8 cores x 4 stripes; every unit gets an identical degree profile
    (padded), so all cores run one identical SPMD program.
  - Edge tensors live in "stripe-feature-major slot layout": a (128, S)
    array per core; partition 32*q+c holds feature c of stripe q; the free
    axis enumerates padded per-node edge slots (nodes sorted by degree, so
    slot runs have uniform depth d and segment-sum = one strided DVE reduce).
  - Per-edge linear = one block-diagonal 128x128 matmul per 512 columns.
  - dst-gather: indirect DMA gathers H[dst] rows (natural-major) which are
    folded into the feature-major PSUM accumulator via identity-matmul
    transposes (one per 128 columns).
  - segment-sums are core-local (each core owns all edges of its nodes);
    the only collectives are two AllGathers of the (N,32) H tables.
"""

import os
import numpy as np
import ml_dtypes

import concourse.bass as bass
import concourse.bacc as bacc
import concourse.mybir as mybir
import concourse.tile as tile
from concourse.bass_utils import run_bass_kernel_spmd

BF16 = ml_dtypes.bfloat16

N_NODES = 100000
N_EDGES = 2400000
F = 32
NCLUST = 30
CORES = 8
STRIPES = 4
UNITS = CORES * STRIPES          # 32
J_UNIT = 3200                    # nodes per unit (multiple of 128)
J_CORE = STRIPES * J_UNIT        # 12800
N_REL = CORES * J_CORE           # 102400 relabeled nodes (>= N_NODES)
NB_N = J_UNIT // 128             # 25 node blocks per stripe
SC = 2048                        # superchunk columns
PSC = 512                        # psum chunk columns
POISON_ID = N_REL - 1            # relabeled id whose H row is -1e9
TRASH_J = J_UNIT                 # agg column for padding reductions
AGG_W = J_UNIT + 128

LAST_RESULTS = None              # BassKernelResults of the last run (for test.py)
LAST_NC = None
LAST_IN_MAPS = None
LAST_PLANS = None


# ----------------------------------------------------------------------------
# Host-side preparation
# ----------------------------------------------------------------------------

def _block_diag4(w):
    """(32,32) -> (128,128) with 4 copies of w on the diagonal."""
    out = np.zeros((128, 128), np.float32)
    for q in range(4):
        out[32 * q:32 * q + 32, 32 * q:32 * q + 32] = w
    return out


def _pack_layout(D_prof):
    """Assign slot columns to nodes; build reduce & G-add segment plans.

    Returns (S, col_start, red_segs, gadd_segs):
      red_segs[sc]   = [(lc, nn, d, jj)]  : reduce cols [lc, lc+nn*d) of the
                       superchunk into agg[:, jj:jj+nn]  (jj==TRASH_J for pads)
      gadd_segs[sc][c] = [(lc, nn, d, j0)]: per 512-col psum chunk, add
                       G[:, j0+i] to slot cols (lc local to the psum chunk)
    """
    segs = []  # (col0, j0 (-1 pad), nn, d) ; each within one superchunk
    col_start = np.zeros(J_UNIT, np.int64)
    cur = 0
    pend = None

    def flush():
        nonlocal pend
        if pend is not None:
            segs.append(pend)
            pend = None

    for j in range(J_UNIT):
        d = int(D_prof[j])
        if d == 0:
            continue
        room = SC - (cur % SC)
        if d > room:
            flush()
            segs.append((cur, -1, 1, room))
            cur += room
        col_start[j] = cur
        if (pend is not None and pend[3] == d and pend[1] + pend[2] == j
                and pend[0] // SC == cur // SC):
            pend = (pend[0], pend[1], pend[2] + 1, d)
        else:
            flush()
            pend = (cur, j, 1, d)
        cur += d
    flush()
    tail = (-cur) % SC
    if tail:
        segs.append((cur, -1, 1, tail))
        cur += tail
    S = cur
    nsc = S // SC

    red_segs = [[] for _ in range(nsc)]
    for (c0, j0, nn, d) in segs:
        sc = c0 // SC
        jj = TRASH_J if j0 < 0 else j0
        red_segs[sc].append((c0 - sc * SC, nn, d, jj))

    gadd_segs = [[[] for _ in range(SC // PSC)] for _ in range(nsc)]
    for (c0, j0, nn, d) in segs:
        end = c0 + nn * d
        pos = c0
        while pos < end:
            e = min(end, (pos // PSC + 1) * PSC)
            sc, ch = pos // SC, (pos % SC) // PSC
            base = sc * SC + ch * PSC
            lst = gadd_segs[sc][ch]
            i0, o0 = divmod(pos - c0, d)
            jj = 0 if j0 < 0 else j0 + i0
            head = min(d - o0, e - pos)
            lst.append((pos - base, 1, head, jj))
            pos += head
            whole = (e - pos) // d
            if whole > 0:
                lst.append((pos - base, whole, d, 0 if j0 < 0 else j0 + i0 + 1))
                pos += whole * d
            if pos < e:
                lst.append((pos - base, 1, e - pos,
                            0 if j0 < 0 else j0 + i0 + 1 + whole))
                pos = e
    return S, col_start, red_segs, gadd_segs


def _prepare(inp):
    src = np.asarray(inp["src"], np.int64)
    dst = np.asarray(inp["dst"], np.int64)
    edge_vals = np.asarray(inp["edge_vals"], np.float32)
    D = np.asarray(inp["D"], np.float32)

    deg = np.bincount(src, minlength=N_NODES)
    order = np.argsort(-deg, kind="stable")
    rank = np.empty(N_NODES, np.int64)
    rank[order] = np.arange(N_NODES)

    degs_sorted = np.zeros(N_REL, np.int64)
    degs_sorted[:N_NODES] = deg[order]
    D_prof = degs_sorted.reshape(J_UNIT, UNITS)[:, 0]   # max of each 32-group

    S, col_start, red_segs, gadd_segs = _pack_layout(D_prof)
    nsc = S // SC
    nblk = S // 128

    # edge -> (core, stripe, col)
    r_src = rank[src]
    u_src = r_src % UNITS
    j_src = r_src // UNITS
    k_src = u_src % CORES
    q_src = u_src // CORES
    perm = np.argsort(r_src, kind="stable")
    rs = r_src[perm]
    newgrp = np.r_[True, rs[1:] != rs[:-1]]
    starts = np.flatnonzero(newgrp)
    lens = np.diff(np.r_[starts, N_EDGES])
    cnt = np.empty(N_EDGES, np.int64)
    cnt[perm] = np.arange(N_EDGES) - np.repeat(starts, lens)
    col = col_start[j_src] + cnt

    r_dst = rank[dst]
    u_dst = r_dst % UNITS
    g_dst = (u_dst % CORES) * J_CORE + (u_dst // CORES) * J_UNIT + r_dst // UNITS

    evt0 = np.zeros((CORES, STRIPES, F, S), BF16)
    evt0[k_src, q_src, :, col] = edge_vals.astype(BF16)
    evt0 = evt0.reshape(CORES, 128, S)

    dsti = np.full((CORES, 128, 4 * nblk), POISON_ID, np.int32)
    dsti[k_src, col % 128, 4 * (col // 128) + q_src] = g_dst.astype(np.int32)

    # recipD in stripe layout (dummies -> 0)
    u_grid = np.arange(UNITS)
    r_grid = np.arange(J_UNIT)[None, :] * UNITS + u_grid[:, None]   # (32,3200)
    valid = r_grid < N_NODES
    node_grid = np.where(valid, order[np.minimum(r_grid, N_NODES - 1)], 0)
    rd = np.where(valid, 1.0 / D[node_grid], 0.0).astype(np.float32)
    recipD = rd.reshape(STRIPES, CORES, J_UNIT).transpose(1, 0, 2)  # u = q*8+k
    recipD = np.repeat(recipD[:, :, None, :], F, axis=2).reshape(CORES, 128, J_UNIT)

    # weights
    e1_pw = np.asarray(inp["e1_pw"], np.float32)
    e2_pw = np.asarray(inp["e2_pw"], np.float32)
    A1 = 0.5 * (e1_pw[:, :F] + e1_pw[:, F:])
    B1 = 0.5 * (e1_pw[:, F:] - e1_pw[:, :F])
    A2 = 0.5 * (e2_pw[:, :F] + e2_pw[:, F:])
    B2 = 0.5 * (e2_pw[:, F:] - e2_pw[:, :F])

    def t4(w):                       # (32,32) -> (128,32) replicated per stripe
        return np.tile(np.ascontiguousarray(w), (4, 1)).astype(np.float32)

    n3_pw = np.asarray(inp["n3_pw"], np.float32)
    n3_sw = np.asarray(inp["n3_sw"], np.float32)
    n3pw_pad = np.zeros((F, F), np.float32)
    n3pw_pad[:, :NCLUST] = n3_pw.T
    n3sw_pad = np.zeros((F, F), np.float32)
    n3sw_pad[:, :NCLUST] = n3_sw.T

    # pad columns get a large-negative bias so softmax ignores them; keep it
    # small enough that the ACT Exp LUT stays in a well-defined range
    bias3 = np.full(F, -30.0, np.float32)
    bias3[:NCLUST] = np.asarray(inp["n3_pb"]) + np.asarray(inp["n3_sb"])

    common = {
        "recipd": None,  # per-core
        "bdw1": _block_diag4(np.asarray(inp["e1_sw"], np.float32).T).astype(BF16),
        "bdw2": _block_diag4(np.asarray(inp["e2_sw"], np.float32).T).astype(BF16),
        "bdn1": _block_diag4(np.asarray(inp["n1_pw"], np.float32).T),
        "bdn2pw": _block_diag4(np.asarray(inp["n2_pw"], np.float32).T),
        "bdn2sw": _block_diag4(np.asarray(inp["n2_sw"], np.float32).T),
        "bda1": _block_diag4(A1.T),
        "bda2": _block_diag4(A2.T),
        "b1t4": t4(B1.T),
        "b2t4": t4(B2.T),
        "n3pw": t4(n3pw_pad),
        "n3sw": t4(n3sw_pad),
        "ident": np.eye(128, dtype=np.float32),
        "c1": np.tile(np.asarray(inp["e1_pb"]) + np.asarray(inp["e1_sb"]), 4)
                .astype(np.float32)[:, None],
        "c2": np.tile(np.asarray(inp["e2_pb"]) + np.asarray(inp["e2_sb"]), 4)
                .astype(np.float32)[:, None],
        "bias1": np.tile(np.asarray(inp["n1_pb"]) + np.asarray(inp["n1_sb"]), 4)
                   .astype(np.float32)[:, None],
        "bias2": np.tile(np.asarray(inp["n2_pb"]) + np.asarray(inp["n2_sb"]), 4)
                   .astype(np.float32)[:, None],
        "bias3": np.tile(bias3, (128, 1)),
        "poison": np.full((1, F), -1e9, np.float32),
    }
    in_maps = []
    for k in range(CORES):
        m = dict(common)
        m["evt0"] = np.ascontiguousarray(evt0[k])
        m["dsti"] = np.ascontiguousarray(dsti[k])
        m["recipd"] = np.ascontiguousarray(recipD[k])
        in_maps.append(m)

    plans = dict(S=S, nsc=nsc, nblk=nblk, red_segs=red_segs, gadd_segs=gadd_segs)
    unmap = dict(node_grid=node_grid, valid=valid)
    return in_maps, plans, unmap


# ----------------------------------------------------------------------------
# Bass program
# ----------------------------------------------------------------------------

def _build(plans):
    ABL = set(filter(None, os.environ.get("KERNEL_ABLATE", "").split(",")))
    dt = mybir.dt
    S, nsc, nblk = plans["S"], plans["nsc"], plans["nblk"]
    red_segs, gadd_segs = plans["red_segs"], plans["gadd_segs"]
    AX = mybir.AxisListType.X
    ADD = mybir.AluOpType.add
    MUL = mybir.AluOpType.mult
    MAX = mybir.AluOpType.max
    RELU = mybir.ActivationFunctionType.Relu
    EXP = mybir.ActivationFunctionType.Exp

    nc = bacc.Bacc(num_devices=CORES)

    EVT0 = nc.declare_dram_parameter("evt0", [128, S], dt.bfloat16, isOutput=False)
    DSTI = nc.declare_dram_parameter("dsti", [128, 4 * nblk], dt.int32, isOutput=False)
    RECIPD = nc.declare_dram_parameter("recipd", [128, J_UNIT], dt.float32, isOutput=False)
    ins_f32 = {}
    for name, shp in [("bdn1", [128, 128]), ("bdn2pw", [128, 128]),
                      ("bdn2sw", [128, 128]), ("bda1", [128, 128]),
                      ("bda2", [128, 128]), ("b1t4", [128, 32]),
                      ("b2t4", [128, 32]), ("n3pw", [128, 32]),
                      ("n3sw", [128, 32]), ("c1", [128, 1]), ("c2", [128, 1]),
                      ("bias1", [128, 1]), ("bias2", [128, 1]),
                      ("bias3", [128, 32]), ("ident", [128, 128]),
                      ("poison", [1, F])]:
        ins_f32[name] = nc.declare_dram_parameter(name, shp, dt.float32, isOutput=False)
    ins_bf = {}
    for name, shp in [("bdw1", [128, 128]), ("bdw2", [128, 128])]:
        ins_bf[name] = nc.declare_dram_parameter(name, shp, dt.bfloat16, isOutput=False)
    OUT = nc.declare_dram_parameter("out", [J_CORE, F], dt.float32, isOutput=True)

    EVT1 = nc.dram_tensor("evt1", [128, S], dt.bfloat16)
    HSL = [nc.dram_tensor(f"h{i}sl", [J_CORE, F], dt.float32) for i in (1, 2)]
    HF = [nc.dram_tensor(f"h{i}f", [N_REL, F], dt.float32, addr_space="Shared")
          for i in (1, 2)]
    HFL = [nc.dram_tensor(f"h{i}fl", [N_REL, F], dt.float32) for i in (1, 2)]

    debug = bool(int(os.environ.get("KERNEL_DEBUG", "0")))
    if debug:
        DBG = {
            "dbg_agg1": nc.declare_dram_parameter("dbg_agg1", [128, AGG_W], dt.float32, isOutput=True),
            "dbg_xh1": nc.declare_dram_parameter("dbg_xh1", [128, J_UNIT], dt.float32, isOutput=True),
            "dbg_h1f": nc.declare_dram_parameter("dbg_h1f", [N_REL, F], dt.bfloat16, isOutput=True),
            "dbg_agg2": nc.declare_dram_parameter("dbg_agg2", [128, AGG_W], dt.float32, isOutput=True),
            "dbg_h2f": nc.declare_dram_parameter("dbg_h2f", [N_REL, F], dt.bfloat16, isOutput=True),
            "dbg_agg3": nc.declare_dram_parameter("dbg_agg3", [128, AGG_W], dt.float32, isOutput=True),
        }

    with tile.TileContext(nc) as tc:
        with (tc.tile_pool(name="const", bufs=1) as cp,
              tc.tile_pool(name="pers", bufs=1) as pp,
              tc.tile_pool(name="sA", bufs=2) as sA,
              tc.tile_pool(name="sE", bufs=2) as sE,
              tc.tile_pool(name="sS", bufs=2) as sS,
              tc.tile_pool(name="pz", bufs=4, space="PSUM") as pz,
              tc.tile_pool(name="pn", bufs=2, space="PSUM") as pn,
              tc.tile_pool(name="psml", bufs=2, space="PSUM") as psml):

            # ---- load constants into SBUF
            csb = {}
            for name, hdl in list(ins_f32.items()) + list(ins_bf.items()):
                t = cp.tile(list(hdl.shape), hdl.dtype, tag=name)
                nc.sync.dma_start(out=t[:], in_=hdl[:])
                csb[name] = t
            recipd_sb = cp.tile([128, J_UNIT], dt.float32, name="recipd", tag="recipd")
            nc.sync.dma_start(out=recipd_sb[:], in_=RECIPD[:])
            zcol = cp.tile([128, 1], dt.float32, name="zcol", tag="zcol")
            nc.vector.memset(zcol[:], 0.0)
            dsti_sb = cp.tile([128, 4 * nblk], dt.int32, name="dsti", tag="dsti")
            nc.sync.dma_start(out=dsti_sb[:], in_=DSTI[:])


            # ---- persistent SBUF state
            agg = [pp.tile([128, AGG_W], dt.float32, name=f"agg{i}", tag=f"agg{i}") for i in range(3)]
            Xh = [pp.tile([128, J_UNIT], dt.float32, name=f"xh{i}", tag=f"xh{i}") for i in range(2)]
            G = [pp.tile([128, J_UNIT], dt.float32, name=f"g{i}", tag=f"g{i}") for i in range(2)]
            def reduce_into(agg_t, src_tile, sc):
                for (lc, nn, d, jj) in red_segs[sc]:
                    nc.vector.tensor_reduce(
                        out=agg_t[:, jj:jj + nn],
                        in_=src_tile[:, lc:lc + nn * d].rearrange(
                            "p (n d) -> p n d", d=d),
                        axis=AX, op=ADD)

            # ================= node conv (shared helper) =====================
            def node_layer(agg_t, lin, xh_out, bias):
                """xh_out = relu(sum_i lin_i + bias); lin = [(bd_w, rhs_tile)]"""
                nc.vector.tensor_tensor(out=agg_t[:, :J_UNIT], in0=agg_t[:, :J_UNIT],
                                        in1=recipd_sb[:], op=MUL)
                for c0 in range(0, J_UNIT, PSC):
                    w = min(PSC, J_UNIT - c0)
                    ps = pn.tile([128, PSC], dt.float32, name="nodeps", tag="nodeps")
                    for i, (bd, rhs) in enumerate(lin):
                        nc.tensor.matmul(out=ps[:, :w], lhsT=bd[:],
                                         rhs=rhs[:, c0:c0 + w],
                                         start=(i == 0), stop=(i == len(lin) - 1))
                    nc.scalar.activation(out=xh_out[:, c0:c0 + w], in_=ps[:, :w],
                                         func=RELU, bias=bias[:])

            def g_table(xh, bda, g_out):
                for c0 in range(0, J_UNIT, PSC):
                    w = min(PSC, J_UNIT - c0)
                    ps = pn.tile([128, PSC], dt.float32, name="nodeps", tag="nodeps")
                    nc.tensor.matmul(out=ps[:, :w], lhsT=bda[:],
                                     rhs=xh[:, c0:c0 + w], start=True, stop=True)
                    nc.vector.tensor_copy(out=g_out[:, c0:c0 + w], in_=ps[:, :w])

            def h_table(xh, bt4, hsl, hf, hfl):
                for q in range(4):
                    hst = sS.tile([128, NB_N * F], dt.float32, name="hst", tag="hst")
                    for p0 in range(0, NB_N, 16):
                        pw = min(16, NB_N - p0)
                        ps = psml.tile([128, 16 * F], dt.float32, name="hps",
                                       tag="hps")
                        for i in range(pw):
                            blk = p0 + i
                            nc.tensor.matmul(
                                out=ps[:, i * F:(i + 1) * F],
                                lhsT=xh[32 * q:32 * q + 32,
                                        blk * 128:(blk + 1) * 128],
                                rhs=bt4[32 * q:32 * q + 32, :],
                                start=True, stop=True, tile_position=(32 * q, 0))
                        nc.vector.tensor_copy(
                            out=hst[:, p0 * F:(p0 + pw) * F],
                            in_=ps[:, :pw * F])
                    nc.sync.dma_start(
                        out=hsl[q * J_UNIT:(q + 1) * J_UNIT, :].rearrange(
                            "(b p) c -> p b c", p=128),
                        in_=hst[:].rearrange("p (b c) -> p b c", c=F))
                if "nocoll" in ABL:
                    nc.sync.dma_start(out=hf[:J_CORE, :], in_=hsl[:])
                else:
                    nc.gpsimd.collective_compute(
                        "AllGather", mybir.AluOpType.bypass,
                        ins=[hsl[:]], outs=[hf[:]],
                        replica_groups=[list(range(CORES))])
                nc.sync.dma_start(out=hf[POISON_ID:POISON_ID + 1, :],
                                  in_=csb["poison"][:1, :])
                if "localhf" in ABL:
                    nc.sync.dma_start(out=hfl[:], in_=hf[:])
                    return hfl
                return hf

            # ================= edge pass (shared helper) =====================
            def edge_pass(src_dram, hf, bdw, g_t, c_bias, agg_out, store):
                for sc in range(nsc):
                    ev = sE.tile([128, SC], dt.bfloat16, name="evs", tag="evs", bufs=4)
                    if "noev" not in ABL:
                        nc.sync.dma_start(out=ev[:],
                                          in_=src_dram[:, sc * SC:(sc + 1) * SC])
                    hg = sE.tile([128, SC], dt.float32, name="hg", tag="hg")
                    if "nogather" not in ABL and "nohg" not in ABL:
                        # HW SWDGE pairs one offset per contiguous dest run
                        # per partition (see probe_gather.py), so each 32-col
                        # band needs its own instruction.
                        for kk in range(sc * 64, (sc + 1) * 64):
                            nc.gpsimd.indirect_dma_start(
                                out=hg[:, (kk % 64) * 32:(kk % 64) * 32 + 32],
                                out_offset=None,
                                in_=hf[:],
                                in_offset=bass.IndirectOffsetOnAxis(
                                    ap=dsti_sb[:, kk:kk + 1], axis=0))
                    evo = sE.tile([128, SC], dt.bfloat16, name="evo", tag="evo")
                    for c in range(SC // PSC):
                        zp = pz.tile([128, PSC], dt.float32, name="zp", tag="zp")
                        nc.tensor.matmul(out=zp[:], lhsT=bdw[:],
                                         rhs=ev[:, c * PSC:(c + 1) * PSC],
                                         start=True, stop=("nohg" in ABL))
                        if "nohg" not in ABL:
                            for b in range(PSC // 128):
                                bb = 4 * c + b
                                nc.tensor.matmul(
                                    out=zp[:, b * 128:(b + 1) * 128],
                                    lhsT=hg[:, bb * 128:(bb + 1) * 128],
                                    rhs=csb["ident"][:],
                                    start=False, stop=(b == PSC // 128 - 1))
                        for (lc, nn, d, j0) in gadd_segs[sc][c]:
                            v = zp[:, lc:lc + nn * d].rearrange(
                                "p (n d) -> p n d", d=d)
                            g = g_t[:, j0:j0 + nn].unsqueeze(2).to_broadcast(
                                [128, nn, d])
                            nc.vector.tensor_tensor(out=v, in0=v, in1=g, op=ADD)
                        nc.scalar.activation(out=evo[:, c * PSC:(c + 1) * PSC],
                                             in_=zp[:], func=RELU, bias=c_bias[:])
                    if store is not None and "nostore" not in ABL:
                        nc.sync.dma_start(out=store[:, sc * SC:(sc + 1) * SC],
                                          in_=evo[:])
                    if "nored" not in ABL:
                        reduce_into(agg_out, evo, sc)

            UPTO = os.environ.get("KERNEL_UPTO", "full")

            def body():
              # ============ pass A: agg1 = segsum(edge_vals) =================
              # The strided (3-dim AP) TensorReduce encoding has room for only
              # one sync wait; bounce the DMA'd tile through a same-engine DVE
              # copy so the reduces carry no cross-engine waits at all.
              for a in agg:
                  nc.vector.memset(a[:], 0.0)
              if UPTO != "full":
                  for x in (*Xh, *G):
                      nc.vector.memset(x[:], 0.0)
              passa_reads = 0 if "noev" in ABL else (
                  2 if "dblev" in ABL else 1)
              for sc in range(nsc):
                  for _rd in range(passa_reads):
                      t = sE.tile([128, SC], dt.bfloat16, name="ev0a", tag="evs",
                                  bufs=4)
                      nc.sync.dma_start(out=t[:],
                                        in_=EVT0[:, sc * SC:(sc + 1) * SC])
                  if "nored" not in ABL:
                      tc2 = sE.tile([128, SC], dt.bfloat16, name="ev0c", tag="evo")
                      nc.vector.tensor_copy(out=tc2[:], in_=t[:])
                      reduce_into(agg[0], tc2, sc)
              if debug:
                  nc.sync.dma_start(out=DBG["dbg_agg1"][:], in_=agg[0][:])
              if UPTO == "a":
                  return final_softmax()

              # layer 1 node conv (X input is zeroed in reference -> bias only)
              node_layer(agg[0], [(csb["bdn1"], agg[0])], Xh[0], csb["bias1"])
              g_table(Xh[0], csb["bda1"], G[0])
              hf1 = h_table(Xh[0], csb["b1t4"], HSL[0], HF[0], HFL[0])
              if debug:
                  nc.sync.dma_start(out=DBG["dbg_xh1"][:], in_=Xh[0][:])
                  nc.sync.dma_start(out=DBG["dbg_h1f"][:], in_=HF[0][:])
              if UPTO == "n1":
                  return final_softmax()

              # ============ pass B: edge conv 1 (+store ev1) + segsum ========
              edge_pass(EVT0, hf1, csb["bdw1"], G[0], csb["c1"], agg[1], EVT1)
              if UPTO == "b":
                  return final_softmax()

              # layer 2 node conv
              if debug:
                  nc.sync.dma_start(out=DBG["dbg_agg2"][:], in_=agg[1][:])
              node_layer(agg[1], [(csb["bdn2pw"], agg[1]), (csb["bdn2sw"], Xh[0])],
                         Xh[1], csb["bias2"])
              g_table(Xh[1], csb["bda2"], G[1])
              hf2 = h_table(Xh[1], csb["b2t4"], HSL[1], HF[1], HFL[1])
              if debug:
                  nc.sync.dma_start(out=DBG["dbg_h2f"][:], in_=HF[1][:])
              if UPTO == "n2":
                  return final_softmax()

              # ============ pass C: edge conv 2 + segsum =====================
              edge_pass(EVT1, hf2, csb["bdw2"], G[1], csb["c2"], agg[2], None)

              # ============ final node layer + softmax =======================
              if debug:
                  nc.sync.dma_start(out=DBG["dbg_agg3"][:], in_=agg[2][:])
              nc.vector.tensor_tensor(out=agg[2][:, :J_UNIT], in0=agg[2][:, :J_UNIT],
                                      in1=recipd_sb[:], op=MUL)
              final_softmax()

            def final_softmax():
              if "nosm" in ABL:
                  nc.sync.dma_start(
                      out=OUT[:].rearrange("(b p) c -> p b c", p=128),
                      in_=agg[2][:, :100 * F].rearrange("p (b c) -> p b c", c=F))
                  return
              for q in range(4):
                outst = sS.tile([128, NB_N * F], dt.float32, name="outst", tag="outst")
                for p0 in range(0, NB_N, 16):
                    pw = min(16, NB_N - p0)
                    fx = psml.tile([128, 16 * F], dt.float32, name="hps", tag="hps")
                    for i in range(pw):
                        blk = p0 + i
                        nc.tensor.matmul(
                            out=fx[:, i * F:(i + 1) * F],
                            lhsT=agg[2][32 * q:32 * q + 32,
                                        blk * 128:(blk + 1) * 128],
                            rhs=csb["n3pw"][32 * q:32 * q + 32, :],
                            start=True, stop=False, tile_position=(32 * q, 0))
                        nc.tensor.matmul(
                            out=fx[:, i * F:(i + 1) * F],
                            lhsT=Xh[1][32 * q:32 * q + 32,
                                       blk * 128:(blk + 1) * 128],
                            rhs=csb["n3sw"][32 * q:32 * q + 32, :],
                            start=False, stop=True, tile_position=(32 * q, 0))
                    W = pw * F
                    e = sS.tile([128, 16 * F], dt.float32, name="sm_e", tag="sm_e")
                    nc.vector.tensor_tensor(
                        out=e[:, :W].rearrange("p (n f) -> p n f", f=F),
                        in0=fx[:, :W].rearrange("p (n f) -> p n f", f=F),
                        in1=csb["bias3"][:].unsqueeze(1).to_broadcast(
                            [128, pw, F]),
                        op=ADD)
                    nm = sS.tile([128, 16], dt.float32, name="sm_m", tag="sm_m")
                    nc.vector.tensor_reduce(
                        out=nm[:, :pw],
                        in_=e[:, :W].rearrange("p (n f) -> p n f", f=F),
                        axis=AX, op=MAX, negate=True)
                    nc.vector.tensor_tensor(
                        out=e[:, :W].rearrange("p (n f) -> p n f", f=F),
                        in0=e[:, :W].rearrange("p (n f) -> p n f", f=F),
                        in1=nm[:, :pw].unsqueeze(2).to_broadcast([128, pw, F]),
                        op=ADD)
                    ex = sS.tile([128, 16 * F], dt.float32, name="sm_x", tag="sm_x")
                    nc.scalar.activation(out=ex[:, :W], in_=e[:, :W], func=EXP,
                                         bias=zcol[:])
                    sm = sS.tile([128, 16], dt.float32, name="sm_s", tag="sm_s")
                    nc.vector.tensor_reduce(
                        out=sm[:, :pw],
                        in_=ex[:, :W].rearrange("p (n f) -> p n f", f=F),
                        axis=AX, op=ADD)
                    rc = sS.tile([128, 16], dt.float32, name="sm_r", tag="sm_r")
                    nc.vector.reciprocal(out=rc[:, :pw], in_=sm[:, :pw])
                    nc.vector.tensor_tensor(
                        out=outst[:, p0 * F:p0 * F + W].rearrange(
                            "p (n f) -> p n f", f=F),
                        in0=ex[:, :W].rearrange("p (n f) -> p n f", f=F),
                        in1=rc[:, :pw].unsqueeze(2).to_broadcast([128, pw, F]),
                        op=MUL)
                nc.sync.dma_start(
                    out=OUT[q * J_UNIT:(q + 1) * J_UNIT, :].rearrange(
                        "(b p) c -> p b c", p=128),
                    in_=outst[:].rearrange("p (b c) -> p b c", c=F))

            for _rep in range(int(os.environ.get("KERNEL_REPS", "1"))):
                body()
    nc.finalize()
    return nc


# ----------------------------------------------------------------------------
# Entry point
# ----------------------------------------------------------------------------

def build_null(plans, scratch=False):
    """Same I/O signature, minimal work - for wall-clock baseline."""
    dt = mybir.dt
    S, nblk, nsc = plans["S"], plans["nblk"], plans["nsc"]
    nc = bacc.Bacc(num_devices=CORES)
    nc.declare_dram_parameter("evt0", [128, S], dt.bfloat16, isOutput=False)
    nc.declare_dram_parameter("dsti", [128, 4 * nblk], dt.int32, isOutput=False)
    RECIPD = nc.declare_dram_parameter("recipd", [128, J_UNIT], dt.float32, isOutput=False)
    for name, shp in [("bdn1", [128, 128]), ("bdn2pw", [128, 128]),
                      ("bdn2sw", [128, 128]), ("bda1", [128, 128]),
                      ("bda2", [128, 128]), ("b1t4", [128, 32]),
                      ("b2t4", [128, 32]), ("n3pw", [128, 32]),
                      ("n3sw", [128, 32]), ("c1", [128, 1]), ("c2", [128, 1]),
                      ("bias1", [128, 1]), ("bias2", [128, 1]),
                      ("bias3", [128, 32]), ("ident", [128, 128]),
                      ("poison", [1, F])]:
        nc.declare_dram_parameter(name, shp, dt.float32, isOutput=False)
    for name, shp in [("bdw1", [128, 128]), ("bdw2", [128, 128])]:
        nc.declare_dram_parameter(name, shp, dt.bfloat16, isOutput=False)
    OUT = nc.declare_dram_parameter("out", [J_CORE, F], dt.float32, isOutput=True)
    NSL = nc.dram_tensor("nullsl", [8, 4], dt.float32)
    NFL = nc.dram_tensor("nullfl", [64, 4], dt.float32, addr_space="Shared")
    scr = []
    if scratch:
        scr.append(nc.dram_tensor("evt1", [128, S], dt.bfloat16))
        for i in (1, 2):
            scr.append(nc.dram_tensor(f"h{i}sl", [J_CORE, F], dt.bfloat16))
            scr.append(nc.dram_tensor(f"h{i}f", [N_REL, F], dt.bfloat16,
                                      addr_space="Shared"))
            scr.append(nc.dram_tensor(f"h{i}fl", [N_REL, F], dt.bfloat16))
    with tile.TileContext(nc) as tc:
        with tc.tile_pool(name="sb", bufs=1) as sb:
            t = sb.tile([128, J_UNIT], dt.float32, name="t")
            nc.sync.dma_start(out=t[:], in_=RECIPD[:])
            if scr:
                tb = sb.tile([128, 32], dt.bfloat16, name="tb")
                nc.vector.tensor_copy(out=tb[:], in_=t[:, :32])
                for s in scr:
                    nc.sync.dma_start(out=s[:1, :32], in_=tb[:1, :32])
            # tiny collective so the 8-core mesh stays in sync (matches the
            # real kernel's use of the collective path; ~Âµs of extra work)
            nc.sync.dma_start(out=NSL[:], in_=t[:8, :4])
            nc.gpsimd.collective_compute(
                "AllGather", mybir.AluOpType.bypass,
                ins=[NSL[:]], outs=[NFL[:]],
                replica_groups=[list(range(CORES))])
            nc.sync.dma_start(out=OUT[:].rearrange("(a p) c -> p a c", p=128),
                              in_=t[:, :100 * F].rearrange("p (a c) -> p a c", c=F))
    nc.finalize()
    return nc


def kernel(**inputs):
    global LAST_RESULTS, LAST_NC, LAST_IN_MAPS, LAST_PLANS
    in_maps, plans, unmap = _prepare(inputs)
    nc = _build(plans)
    LAST_NC, LAST_IN_MAPS, LAST_PLANS = nc, in_maps, plans
    trace = bool(int(os.environ.get("KERNEL_TRACE", "0")))
    res = run_bass_kernel_spmd(nc, in_maps, list(range(CORES)), trace=trace)
    LAST_RESULTS = res

    node_grid, valid = unmap["node_grid"], unmap["valid"]
    out = np.zeros((N_NODES, NCLUST), np.float32)
    for k in range(CORES):
        rk = np.asarray(res.results[k]["out"], np.float32)
        for q in range(4):
            u = q * CORES + k
            v = valid[u]
            out[node_grid[u, v]] = rk[q * J_UNIT:(q + 1) * J_UNIT][v][:, :NCLUST]
    return out



# revision 36
# speedup vs baseline: 6.2993x; 6.2993x over previous
"""Trainium2 Bass kernel for nn_GCNEdgeBasedEdgeGenCluster (3-layer edge-GCN).

Strategy (8 NeuronCores, src-owner sharding):
  - Nodes are relabeled by (degree-sorted, dealt round-robin) into 32 "units"
    = 8 cores x 4 stripes; every unit gets an identical degree profile
    (padded), so all cores run one identical SPMD program.
  - Edge tensors live in "stripe-feature-major slot layout": a (128, S)
    array per core; partition 32*q+c holds feature c of stripe q; the free
    axis enumerates padded per-node edge slots (nodes sorted by degree, so
    slot runs have uniform depth d and segment-sum = one strided DVE reduce).
  - Per-edge linear = one block-diagonal 128x128 matmul per 512 columns.
  - dst-gather: indirect DMA gathers H[dst] rows (natural-major) which are
    folded into the feature-major PSUM accumulator via identity-matmul
    transposes (one per 128 columns).
  - segment-sums are core-local (each core owns all edges of its nodes);
    the only collectives are two AllGathers of the (N,32) H tables.
"""

import os
import numpy as np
import ml_dtypes

import concourse.bass as bass
import concourse.bacc as bacc
import concourse.mybir as mybir
import concourse.tile as tile
from concourse.bass_utils import run_bass_kernel_spmd

BF16 = ml_dtypes.bfloat16

N_NODES = 100000
N_EDGES = 2400000
F = 32
NCLUST = 30
CORES = 8
STRIPES = 4
UNITS = CORES * STRIPES          # 32
J_UNIT = 3200                    # nodes per unit (multiple of 128)
J_CORE = STRIPES * J_UNIT        # 12800
N_REL = CORES * J_CORE           # 102400 relabeled nodes (>= N_NODES)
NB_N = J_UNIT // 128             # 25 node blocks per stripe
SC = 2048                        # superchunk columns
PSC = 512                        # psum chunk columns
POISON_ID = N_REL - 1            # relabeled id whose H row is -1e9
TRASH_J = J_UNIT                 # agg column for padding reductions
AGG_W = J_UNIT + 128

LAST_RESULTS = None              # BassKernelResults of the last run (for test.py)
LAST_NC = None
LAST_IN_MAPS = None
LAST_PLANS = None


# ----------------------------------------------------------------------------
# Host-side preparation
# ----------------------------------------------------------------------------

def _block_diag4(w):
    """(32,32) -> (128,128) with 4 copies of w on the diagonal."""
    out = np.zeros((128, 128), np.float32)
    for q in range(4):
        out[32 * q:32 * q + 32, 32 * q:32 * q + 32] = w
    return out


def _pack_layout(D_prof):
    """Assign slot columns to nodes; build reduce & G-add segment plans.

    Returns (S, col_start, red_segs, gadd_segs):
      red_segs[sc]   = [(lc, nn, d, jj)]  : reduce cols [lc, lc+nn*d) of the
                       superchunk into agg[:, jj:jj+nn]  (jj==TRASH_J for pads)
      gadd_segs[sc][c] = [(lc, nn, d, j0)]: per 512-col psum chunk, add
                       G[:, j0+i] to slot cols (lc local to the psum chunk)
    """
    segs = []  # (col0, j0 (-1 pad), nn, d) ; each within one superchunk
    col_start = np.zeros(J_UNIT, np.int64)
    cur = 0
    pend = None

    def flush():
        nonlocal pend
        if pend is not None:
            segs.append(pend)
            pend = None

    for j in range(J_UNIT):
        d = int(D_prof[j])
        if d == 0:
            continue
        room = SC - (cur % SC)
        if d > room:
            flush()
            segs.append((cur, -1, 1, room))
            cur += room
        col_start[j] = cur
        if (pend is not None and pend[3] == d and pend[1] + pend[2] == j
                and pend[0] // SC == cur // SC):
            pend = (pend[0], pend[1], pend[2] + 1, d)
        else:
            flush()
            pend = (cur, j, 1, d)
        cur += d
    flush()
    tail = (-cur) % SC
    if tail:
        segs.append((cur, -1, 1, tail))
        cur += tail
    S = cur
    nsc = S // SC

    red_segs = [[] for _ in range(nsc)]
    for (c0, j0, nn, d) in segs:
        sc = c0 // SC
        jj = TRASH_J if j0 < 0 else j0
        red_segs[sc].append((c0 - sc * SC, nn, d, jj))

    gadd_segs = [[[] for _ in range(SC // PSC)] for _ in range(nsc)]
    for (c0, j0, nn, d) in segs:
        end = c0 + nn * d
        pos = c0
        while pos < end:
            e = min(end, (pos // PSC + 1) * PSC)
            sc, ch = pos // SC, (pos % SC) // PSC
            base = sc * SC + ch * PSC
            lst = gadd_segs[sc][ch]
            i0, o0 = divmod(pos - c0, d)
            jj = 0 if j0 < 0 else j0 + i0
            head = min(d - o0, e - pos)
            lst.append((pos - base, 1, head, jj))
            pos += head
            whole = (e - pos) // d
            if whole > 0:
                lst.append((pos - base, whole, d, 0 if j0 < 0 else j0 + i0 + 1))
                pos += whole * d
            if pos < e:
                lst.append((pos - base, 1, e - pos,
                            0 if j0 < 0 else j0 + i0 + 1 + whole))
                pos = e
    return S, col_start, red_segs, gadd_segs


def _prepare(inp):
    src = np.asarray(inp["src"], np.int64)
    dst = np.asarray(inp["dst"], np.int64)
    edge_vals = np.asarray(inp["edge_vals"], np.float32)
    D = np.asarray(inp["D"], np.float32)

    deg = np.bincount(src, minlength=N_NODES)
    order = np.argsort(-deg, kind="stable")
    rank = np.empty(N_NODES, np.int64)
    rank[order] = np.arange(N_NODES)

    degs_sorted = np.zeros(N_REL, np.int64)
    degs_sorted[:N_NODES] = deg[order]
    D_prof = degs_sorted.reshape(J_UNIT, UNITS)[:, 0]   # max of each 32-group

    S, col_start, red_segs, gadd_segs = _pack_layout(D_prof)
    nsc = S // SC
    nblk = S // 128

    # edge -> (core, stripe, col)
    r_src = rank[src]
    u_src = r_src % UNITS
    j_src = r_src // UNITS
    k_src = u_src % CORES
    q_src = u_src // CORES
    perm = np.argsort(r_src, kind="stable")
    rs = r_src[perm]
    newgrp = np.r_[True, rs[1:] != rs[:-1]]
    starts = np.flatnonzero(newgrp)
    lens = np.diff(np.r_[starts, N_EDGES])
    cnt = np.empty(N_EDGES, np.int64)
    cnt[perm] = np.arange(N_EDGES) - np.repeat(starts, lens)
    col = col_start[j_src] + cnt

    r_dst = rank[dst]
    u_dst = r_dst % UNITS
    g_dst = (u_dst % CORES) * J_CORE + (u_dst // CORES) * J_UNIT + r_dst // UNITS

    evt0 = np.zeros((CORES, STRIPES, F, S), BF16)
    evt0[k_src, q_src, :, col] = edge_vals.astype(BF16)
    evt0 = evt0.reshape(CORES, 128, S)

    dsti = np.full((CORES, 128, 4 * nblk), POISON_ID, np.int32)
    dsti[k_src, col % 128, 4 * (col // 128) + q_src] = g_dst.astype(np.int32)

    # recipD in stripe layout (dummies -> 0)
    u_grid = np.arange(UNITS)
    r_grid = np.arange(J_UNIT)[None, :] * UNITS + u_grid[:, None]   # (32,3200)
    valid = r_grid < N_NODES
    node_grid = np.where(valid, order[np.minimum(r_grid, N_NODES - 1)], 0)
    rd = np.where(valid, 1.0 / D[node_grid], 0.0).astype(np.float32)
    recipD = rd.reshape(STRIPES, CORES, J_UNIT).transpose(1, 0, 2)  # u = q*8+k
    recipD = np.repeat(recipD[:, :, None, :], F, axis=2).reshape(CORES, 128, J_UNIT)

    # weights
    e1_pw = np.asarray(inp["e1_pw"], np.float32)
    e2_pw = np.asarray(inp["e2_pw"], np.float32)
    A1 = 0.5 * (e1_pw[:, :F] + e1_pw[:, F:])
    B1 = 0.5 * (e1_pw[:, F:] - e1_pw[:, :F])
    A2 = 0.5 * (e2_pw[:, :F] + e2_pw[:, F:])
    B2 = 0.5 * (e2_pw[:, F:] - e2_pw[:, :F])

    def t4(w):                       # (32,32) -> (128,32) replicated per stripe
        return np.tile(np.ascontiguousarray(w), (4, 1)).astype(np.float32)

    n3_pw = np.asarray(inp["n3_pw"], np.float32)
    n3_sw = np.asarray(inp["n3_sw"], np.float32)
    n3pw_pad = np.zeros((F, F), np.float32)
    n3pw_pad[:, :NCLUST] = n3_pw.T
    n3sw_pad = np.zeros((F, F), np.float32)
    n3sw_pad[:, :NCLUST] = n3_sw.T

    # pad columns get a large-negative bias so softmax ignores them; keep it
    # small enough that the ACT Exp LUT stays in a well-defined range
    bias3 = np.full(F, -30.0, np.float32)
    bias3[:NCLUST] = np.asarray(inp["n3_pb"]) + np.asarray(inp["n3_sb"])

    common = {
        "recipd": None,  # per-core
        "bdw1": _block_diag4(np.asarray(inp["e1_sw"], np.float32).T).astype(BF16),
        "bdw2": _block_diag4(np.asarray(inp["e2_sw"], np.float32).T).astype(BF16),
        "bdn1": _block_diag4(np.asarray(inp["n1_pw"], np.float32).T),
        "bdn2pw": _block_diag4(np.asarray(inp["n2_pw"], np.float32).T),
        "bdn2sw": _block_diag4(np.asarray(inp["n2_sw"], np.float32).T),
        "bda1": _block_diag4(A1.T),
        "bda2": _block_diag4(A2.T),
        "b1t4": t4(B1.T),
        "b2t4": t4(B2.T),
        "n3pw": t4(n3pw_pad),
        "n3sw": t4(n3sw_pad),
        "ident": np.eye(128, dtype=BF16),
        "c1": np.tile(np.asarray(inp["e1_pb"]) + np.asarray(inp["e1_sb"]), 4)
                .astype(np.float32)[:, None],
        "c2": np.tile(np.asarray(inp["e2_pb"]) + np.asarray(inp["e2_sb"]), 4)
                .astype(np.float32)[:, None],
        "bias1": np.tile(np.asarray(inp["n1_pb"]) + np.asarray(inp["n1_sb"]), 4)
                   .astype(np.float32)[:, None],
        "bias2": np.tile(np.asarray(inp["n2_pb"]) + np.asarray(inp["n2_sb"]), 4)
                   .astype(np.float32)[:, None],
        "bias3": np.tile(bias3, (128, 1)),
        "poison": np.full((1, F), -1e9, BF16),
    }
    in_maps = []
    for k in range(CORES):
        m = dict(common)
        m["evt0"] = np.ascontiguousarray(evt0[k])
        m["dsti"] = np.ascontiguousarray(dsti[k])
        m["recipd"] = np.ascontiguousarray(recipD[k])
        in_maps.append(m)

    plans = dict(S=S, nsc=nsc, nblk=nblk, red_segs=red_segs, gadd_segs=gadd_segs)
    unmap = dict(node_grid=node_grid, valid=valid)
    return in_maps, plans, unmap


# ----------------------------------------------------------------------------
# Bass program
# ----------------------------------------------------------------------------

def _build(plans):
    ABL = set(filter(None, os.environ.get("KERNEL_ABLATE", "").split(",")))
    dt = mybir.dt
    S, nsc, nblk = plans["S"], plans["nsc"], plans["nblk"]
    red_segs, gadd_segs = plans["red_segs"], plans["gadd_segs"]
    AX = mybir.AxisListType.X
    ADD = mybir.AluOpType.add
    MUL = mybir.AluOpType.mult
    MAX = mybir.AluOpType.max
    RELU = mybir.ActivationFunctionType.Relu
    EXP = mybir.ActivationFunctionType.Exp

    nc = bacc.Bacc(num_devices=CORES)

    EVT0 = nc.declare_dram_parameter("evt0", [128, S], dt.bfloat16, isOutput=False)
    DSTI = nc.declare_dram_parameter("dsti", [128, 4 * nblk], dt.int32, isOutput=False)
    RECIPD = nc.declare_dram_parameter("recipd", [128, J_UNIT], dt.float32, isOutput=False)
    ins_f32 = {}
    for name, shp in [("bdn1", [128, 128]), ("bdn2pw", [128, 128]),
                      ("bdn2sw", [128, 128]), ("bda1", [128, 128]),
                      ("bda2", [128, 128]), ("b1t4", [128, 32]),
                      ("b2t4", [128, 32]), ("n3pw", [128, 32]),
                      ("n3sw", [128, 32]), ("c1", [128, 1]), ("c2", [128, 1]),
                      ("bias1", [128, 1]), ("bias2", [128, 1]),
                      ("bias3", [128, 32])]:
        ins_f32[name] = nc.declare_dram_parameter(name, shp, dt.float32, isOutput=False)
    ins_bf = {}
    for name, shp in [("bdw1", [128, 128]), ("bdw2", [128, 128]),
                      ("ident", [128, 128]), ("poison", [1, F])]:
        ins_bf[name] = nc.declare_dram_parameter(name, shp, dt.bfloat16, isOutput=False)
    OUT = nc.declare_dram_parameter("out", [J_CORE, F], dt.float32, isOutput=True)

    EVT1 = nc.dram_tensor("evt1", [128, S], dt.bfloat16)
    HSL = [nc.dram_tensor(f"h{i}sl", [J_CORE, F], dt.bfloat16) for i in (1, 2)]
    HF = [nc.dram_tensor(f"h{i}f", [N_REL, F], dt.bfloat16, addr_space="Shared")
          for i in (1, 2)]
    HFL = [nc.dram_tensor(f"h{i}fl", [N_REL, F], dt.bfloat16) for i in (1, 2)]

    debug = bool(int(os.environ.get("KERNEL_DEBUG", "0")))
    if debug:
        DBG = {
            "dbg_agg1": nc.declare_dram_parameter("dbg_agg1", [128, AGG_W], dt.float32, isOutput=True),
            "dbg_xh1": nc.declare_dram_parameter("dbg_xh1", [128, J_UNIT], dt.float32, isOutput=True),
            "dbg_h1f": nc.declare_dram_parameter("dbg_h1f", [N_REL, F], dt.bfloat16, isOutput=True),
            "dbg_agg2": nc.declare_dram_parameter("dbg_agg2", [128, AGG_W], dt.float32, isOutput=True),
            "dbg_h2f": nc.declare_dram_parameter("dbg_h2f", [N_REL, F], dt.bfloat16, isOutput=True),
            "dbg_agg3": nc.declare_dram_parameter("dbg_agg3", [128, AGG_W], dt.float32, isOutput=True),
        }

    with tile.TileContext(nc) as tc:
        with (tc.tile_pool(name="const", bufs=1) as cp,
              tc.tile_pool(name="pers", bufs=1) as pp,
              tc.tile_pool(name="sA", bufs=2) as sA,
              tc.tile_pool(name="sE", bufs=2) as sE,
              tc.tile_pool(name="sS", bufs=2) as sS,
              tc.tile_pool(name="pz", bufs=4, space="PSUM") as pz,
              tc.tile_pool(name="pn", bufs=2, space="PSUM") as pn,
              tc.tile_pool(name="psml", bufs=2, space="PSUM") as psml):

            # ---- load constants into SBUF
            csb = {}
            for name, hdl in list(ins_f32.items()) + list(ins_bf.items()):
                t = cp.tile(list(hdl.shape), hdl.dtype, tag=name)
                nc.sync.dma_start(out=t[:], in_=hdl[:])
                csb[name] = t
            recipd_sb = cp.tile([128, J_UNIT], dt.float32, name="recipd", tag="recipd")
            nc.sync.dma_start(out=recipd_sb[:], in_=RECIPD[:])
            zcol = cp.tile([128, 1], dt.float32, name="zcol", tag="zcol")
            nc.vector.memset(zcol[:], 0.0)
            dsti_sb = cp.tile([128, 4 * nblk], dt.int32, name="dsti", tag="dsti")
            nc.sync.dma_start(out=dsti_sb[:], in_=DSTI[:])


            # ---- persistent SBUF state
            agg = [pp.tile([128, AGG_W], dt.float32, name=f"agg{i}", tag=f"agg{i}") for i in range(3)]
            Xh = [pp.tile([128, J_UNIT], dt.float32, name=f"xh{i}", tag=f"xh{i}") for i in range(2)]
            G = [pp.tile([128, J_UNIT], dt.float32, name=f"g{i}", tag=f"g{i}") for i in range(2)]
            def reduce_into(agg_t, src_tile, sc):
                for (lc, nn, d, jj) in red_segs[sc]:
                    nc.vector.tensor_reduce(
                        out=agg_t[:, jj:jj + nn],
                        in_=src_tile[:, lc:lc + nn * d].rearrange(
                            "p (n d) -> p n d", d=d),
                        axis=AX, op=ADD)

            # ================= node conv (shared helper) =====================
            def node_layer(agg_t, lin, xh_out, bias):
                """xh_out = relu(sum_i lin_i + bias); lin = [(bd_w, rhs_tile)]"""
                nc.vector.tensor_tensor(out=agg_t[:, :J_UNIT], in0=agg_t[:, :J_UNIT],
                                        in1=recipd_sb[:], op=MUL)
                for c0 in range(0, J_UNIT, PSC):
                    w = min(PSC, J_UNIT - c0)
                    ps = pn.tile([128, PSC], dt.float32, name="nodeps", tag="nodeps")
                    for i, (bd, rhs) in enumerate(lin):
                        nc.tensor.matmul(out=ps[:, :w], lhsT=bd[:],
                                         rhs=rhs[:, c0:c0 + w],
                                         start=(i == 0), stop=(i == len(lin) - 1))
                    nc.scalar.activation(out=xh_out[:, c0:c0 + w], in_=ps[:, :w],
                                         func=RELU, bias=bias[:])

            def g_table(xh, bda, g_out):
                for c0 in range(0, J_UNIT, PSC):
                    w = min(PSC, J_UNIT - c0)
                    ps = pn.tile([128, PSC], dt.float32, name="nodeps", tag="nodeps")
                    nc.tensor.matmul(out=ps[:, :w], lhsT=bda[:],
                                     rhs=xh[:, c0:c0 + w], start=True, stop=True)
                    nc.vector.tensor_copy(out=g_out[:, c0:c0 + w], in_=ps[:, :w])

            def h_table(xh, bt4, hsl, hf, hfl):
                for q in range(4):
                    hst = sS.tile([128, NB_N * F], dt.bfloat16, name="hst", tag="hst")
                    for p0 in range(0, NB_N, 16):
                        pw = min(16, NB_N - p0)
                        ps = psml.tile([128, 16 * F], dt.float32, name="hps",
                                       tag="hps")
                        for i in range(pw):
                            blk = p0 + i
                            nc.tensor.matmul(
                                out=ps[:, i * F:(i + 1) * F],
                                lhsT=xh[32 * q:32 * q + 32,
                                        blk * 128:(blk + 1) * 128],
                                rhs=bt4[32 * q:32 * q + 32, :],
                                start=True, stop=True, tile_position=(32 * q, 0))
                        nc.vector.tensor_copy(
                            out=hst[:, p0 * F:(p0 + pw) * F],
                            in_=ps[:, :pw * F])
                    nc.sync.dma_start(
                        out=hsl[q * J_UNIT:(q + 1) * J_UNIT, :].rearrange(
                            "(b p) c -> p b c", p=128),
                        in_=hst[:].rearrange("p (b c) -> p b c", c=F))
                if "nocoll" in ABL:
                    nc.sync.dma_start(out=hf[:J_CORE, :], in_=hsl[:])
                else:
                    nc.gpsimd.collective_compute(
                        "AllGather", mybir.AluOpType.bypass,
                        ins=[hsl[:]], outs=[hf[:]],
                        replica_groups=[list(range(CORES))])
                nc.sync.dma_start(out=hf[POISON_ID:POISON_ID + 1, :],
                                  in_=csb["poison"][:1, :])
                if "localhf" in ABL:
                    nc.sync.dma_start(out=hfl[:], in_=hf[:])
                    return hfl
                return hf

            # ================= edge pass (shared helper) =====================
            def edge_pass(src_dram, hf, bdw, g_t, c_bias, agg_out, store):
                for sc in range(nsc):
                    ev = sE.tile([128, SC], dt.bfloat16, name="evs", tag="evs", bufs=4)
                    if "noev" not in ABL:
                        nc.sync.dma_start(out=ev[:],
                                          in_=src_dram[:, sc * SC:(sc + 1) * SC])
                    hg = sE.tile([128, SC], dt.bfloat16, name="hg", tag="hg")
                    if "nogather" not in ABL and "nohg" not in ABL:
                        # HW SWDGE pairs one offset per contiguous dest run
                        # per partition (see probe_gather.py), so each 32-col
                        # band needs its own instruction.
                        for kk in range(sc * 64, (sc + 1) * 64):
                            nc.gpsimd.indirect_dma_start(
                                out=hg[:, (kk % 64) * 32:(kk % 64) * 32 + 32],
                                out_offset=None,
                                in_=hf[:],
                                in_offset=bass.IndirectOffsetOnAxis(
                                    ap=dsti_sb[:, kk:kk + 1], axis=0))
                    evo = sE.tile([128, SC], dt.bfloat16, name="evo", tag="evo")
                    for c in range(SC // PSC):
                        zp = pz.tile([128, PSC], dt.float32, name="zp", tag="zp")
                        nc.tensor.matmul(out=zp[:], lhsT=bdw[:],
                                         rhs=ev[:, c * PSC:(c + 1) * PSC],
                                         start=True, stop=("nohg" in ABL))
                        if "nohg" not in ABL:
                            for b in range(PSC // 128):
                                bb = 4 * c + b
                                nc.tensor.matmul(
                                    out=zp[:, b * 128:(b + 1) * 128],
                                    lhsT=hg[:, bb * 128:(bb + 1) * 128],
                                    rhs=csb["ident"][:],
                                    start=False, stop=(b == PSC // 128 - 1))
                        for (lc, nn, d, j0) in gadd_segs[sc][c]:
                            v = zp[:, lc:lc + nn * d].rearrange(
                                "p (n d) -> p n d", d=d)
                            g = g_t[:, j0:j0 + nn].unsqueeze(2).to_broadcast(
                                [128, nn, d])
                            nc.vector.tensor_tensor(out=v, in0=v, in1=g, op=ADD)
                        nc.scalar.activation(out=evo[:, c * PSC:(c + 1) * PSC],
                                             in_=zp[:], func=RELU, bias=c_bias[:])
                    if store is not None and "nostore" not in ABL:
                        nc.sync.dma_start(out=store[:, sc * SC:(sc + 1) * SC],
                                          in_=evo[:])
                    if "nored" not in ABL:
                        reduce_into(agg_out, evo, sc)

            UPTO = os.environ.get("KERNEL_UPTO", "full")

            def body():
              # ============ pass A: agg1 = segsum(edge_vals) =================
              # The strided (3-dim AP) TensorReduce encoding has room for only
              # one sync wait; bounce the DMA'd tile through a same-engine DVE
              # copy so the reduces carry no cross-engine waits at all.
              for a in agg:
                  nc.vector.memset(a[:], 0.0)
              if UPTO != "full":
                  for x in (*Xh, *G):
                      nc.vector.memset(x[:], 0.0)
              passa_reads = 0 if "noev" in ABL else (
                  2 if "dblev" in ABL else 1)
              for sc in range(nsc):
                  for _rd in range(passa_reads):
                      t = sE.tile([128, SC], dt.bfloat16, name="ev0a", tag="evs",
                                  bufs=4)
                      nc.sync.dma_start(out=t[:],
                                        in_=EVT0[:, sc * SC:(sc + 1) * SC])
                  if "nored" not in ABL:
                      tc2 = sE.tile([128, SC], dt.bfloat16, name="ev0c", tag="evo")
                      nc.vector.tensor_copy(out=tc2[:], in_=t[:])
                      reduce_into(agg[0], tc2, sc)
              if debug:
                  nc.sync.dma_start(out=DBG["dbg_agg1"][:], in_=agg[0][:])
              if UPTO == "a":
                  return final_softmax()

              # layer 1 node conv (X input is zeroed in reference -> bias only)
              node_layer(agg[0], [(csb["bdn1"], agg[0])], Xh[0], csb["bias1"])
              g_table(Xh[0], csb["bda1"], G[0])
              hf1 = h_table(Xh[0], csb["b1t4"], HSL[0], HF[0], HFL[0])
              if debug:
                  nc.sync.dma_start(out=DBG["dbg_xh1"][:], in_=Xh[0][:])
                  nc.sync.dma_start(out=DBG["dbg_h1f"][:], in_=HF[0][:])
              if UPTO == "n1":
                  return final_softmax()

              # ============ pass B: edge conv 1 (+store ev1) + segsum ========
              edge_pass(EVT0, hf1, csb["bdw1"], G[0], csb["c1"], agg[1], EVT1)
              if UPTO == "b":
                  return final_softmax()

              # layer 2 node conv
              if debug:
                  nc.sync.dma_start(out=DBG["dbg_agg2"][:], in_=agg[1][:])
              node_layer(agg[1], [(csb["bdn2pw"], agg[1]), (csb["bdn2sw"], Xh[0])],
                         Xh[1], csb["bias2"])
              g_table(Xh[1], csb["bda2"], G[1])
              hf2 = h_table(Xh[1], csb["b2t4"], HSL[1], HF[1], HFL[1])
              if debug:
                  nc.sync.dma_start(out=DBG["dbg_h2f"][:], in_=HF[1][:])
              if UPTO == "n2":
                  return final_softmax()

              # ============ pass C: edge conv 2 + segsum =====================
              edge_pass(EVT1, hf2, csb["bdw2"], G[1], csb["c2"], agg[2], None)

              # ============ final node layer + softmax =======================
              if debug:
                  nc.sync.dma_start(out=DBG["dbg_agg3"][:], in_=agg[2][:])
              nc.vector.tensor_tensor(out=agg[2][:, :J_UNIT], in0=agg[2][:, :J_UNIT],
                                      in1=recipd_sb[:], op=MUL)
              final_softmax()

            def final_softmax():
              if "nosm" in ABL:
                  nc.sync.dma_start(
                      out=OUT[:].rearrange("(b p) c -> p b c", p=128),
                      in_=agg[2][:, :100 * F].rearrange("p (b c) -> p b c", c=F))
                  return
              for q in range(4):
                outst = sS.tile([128, NB_N * F], dt.float32, name="outst", tag="outst")
                for p0 in range(0, NB_N, 16):
                    pw = min(16, NB_N - p0)
                    fx = psml.tile([128, 16 * F], dt.float32, name="hps", tag="hps")
                    for i in range(pw):
                        blk = p0 + i
                        nc.tensor.matmul(
                            out=fx[:, i * F:(i + 1) * F],
                            lhsT=agg[2][32 * q:32 * q + 32,
                                        blk * 128:(blk + 1) * 128],
                            rhs=csb["n3pw"][32 * q:32 * q + 32, :],
                            start=True, stop=False, tile_position=(32 * q, 0))
                        nc.tensor.matmul(
                            out=fx[:, i * F:(i + 1) * F],
                            lhsT=Xh[1][32 * q:32 * q + 32,
                                       blk * 128:(blk + 1) * 128],
                            rhs=csb["n3sw"][32 * q:32 * q + 32, :],
                            start=False, stop=True, tile_position=(32 * q, 0))
                    W = pw * F
                    e = sS.tile([128, 16 * F], dt.float32, name="sm_e", tag="sm_e")
                    nc.vector.tensor_tensor(
                        out=e[:, :W].rearrange("p (n f) -> p n f", f=F),
                        in0=fx[:, :W].rearrange("p (n f) -> p n f", f=F),
                        in1=csb["bias3"][:].unsqueeze(1).to_broadcast(
                            [128, pw, F]),
                        op=ADD)
                    nm = sS.tile([128, 16], dt.float32, name="sm_m", tag="sm_m")
                    nc.vector.tensor_reduce(
                        out=nm[:, :pw],
                        in_=e[:, :W].rearrange("p (n f) -> p n f", f=F),
                        axis=AX, op=MAX, negate=True)
                    nc.vector.tensor_tensor(
                        out=e[:, :W].rearrange("p (n f) -> p n f", f=F),
                        in0=e[:, :W].rearrange("p (n f) -> p n f", f=F),
                        in1=nm[:, :pw].unsqueeze(2).to_broadcast([128, pw, F]),
                        op=ADD)
                    ex = sS.tile([128, 16 * F], dt.float32, name="sm_x", tag="sm_x")
                    nc.scalar.activation(out=ex[:, :W], in_=e[:, :W], func=EXP,
                                         bias=zcol[:])
                    sm = sS.tile([128, 16], dt.float32, name="sm_s", tag="sm_s")
                    nc.vector.tensor_reduce(
                        out=sm[:, :pw],
                        in_=ex[:, :W].rearrange("p (n f) -> p n f", f=F),
                        axis=AX, op=ADD)
                    rc = sS.tile([128, 16], dt.float32, name="sm_r", tag="sm_r")
                    nc.vector.reciprocal(out=rc[:, :pw], in_=sm[:, :pw])
                    nc.vector.tensor_tensor(
                        out=outst[:, p0 * F:p0 * F + W].rearrange(
                            "p (n f) -> p n f", f=F),
                        in0=ex[:, :W].rearrange("p (n f) -> p n f", f=F),
                        in1=rc[:, :pw].unsqueeze(2).to_broadcast([128, pw, F]),
                        op=MUL)
                nc.sync.dma_start(
                    out=OUT[q * J_UNIT:(q + 1) * J_UNIT, :].rearrange(
                        "(b p) c -> p b c", p=128),
                    in_=outst[:].rearrange("p (b c) -> p b c", c=F))

            for _rep in range(int(os.environ.get("KERNEL_REPS", "1"))):
                body()
    nc.finalize()
    return nc


# ----------------------------------------------------------------------------
# Entry point
# ----------------------------------------------------------------------------

def build_null(plans, scratch=False):
    """Same I/O signature, minimal work - for wall-clock baseline."""
    dt = mybir.dt
    S, nblk, nsc = plans["S"], plans["nblk"], plans["nsc"]
    nc = bacc.Bacc(num_devices=CORES)
    nc.declare_dram_parameter("evt0", [128, S], dt.bfloat16, isOutput=False)
    nc.declare_dram_parameter("dsti", [128, 4 * nblk], dt.int32, isOutput=False)
    RECIPD = nc.declare_dram_parameter("recipd", [128, J_UNIT], dt.float32, isOutput=False)
    for name, shp in [("bdn1", [128, 128]), ("bdn2pw", [128, 128]),
                      ("bdn2sw", [128, 128]), ("bda1", [128, 128]),
                      ("bda2", [128, 128]), ("b1t4", [128, 32]),
                      ("b2t4", [128, 32]), ("n3pw", [128, 32]),
                      ("n3sw", [128, 32]), ("c1", [128, 1]), ("c2", [128, 1]),
                      ("bias1", [128, 1]), ("bias2", [128, 1]),
                      ("bias3", [128, 32])]:
        nc.declare_dram_parameter(name, shp, dt.float32, isOutput=False)
    for name, shp in [("bdw1", [128, 128]), ("bdw2", [128, 128]),
                      ("ident", [128, 128]), ("poison", [1, 4 * F])]:
        nc.declare_dram_parameter(name, shp, dt.bfloat16, isOutput=False)
    OUT = nc.declare_dram_parameter("out", [J_CORE, F], dt.float32, isOutput=True)
    NSL = nc.dram_tensor("nullsl", [8, 4], dt.float32)
    NFL = nc.dram_tensor("nullfl", [64, 4], dt.float32, addr_space="Shared")
    scr = []
    if scratch:
        scr.append(nc.dram_tensor("evt1", [128, S], dt.bfloat16))
        for i in (1, 2):
            scr.append(nc.dram_tensor(f"h{i}sl", [J_CORE, F], dt.bfloat16))
            scr.append(nc.dram_tensor(f"h{i}f", [N_REL, F], dt.bfloat16,
                                      addr_space="Shared"))
            scr.append(nc.dram_tensor(f"h{i}fl", [N_REL, F], dt.bfloat16))
    with tile.TileContext(nc) as tc:
        with tc.tile_pool(name="sb", bufs=1) as sb:
            t = sb.tile([128, J_UNIT], dt.float32, name="t")
            nc.sync.dma_start(out=t[:], in_=RECIPD[:])
            if scr:
                tb = sb.tile([128, 32], dt.bfloat16, name="tb")
                nc.vector.tensor_copy(out=tb[:], in_=t[:, :32])
                for s in scr:
                    nc.sync.dma_start(out=s[:1, :32], in_=tb[:1, :32])
            # tiny collective so the 8-core mesh stays in sync (matches the
            # real kernel's use of the collective path; ~Âµs of extra work)
            nc.sync.dma_start(out=NSL[:], in_=t[:8, :4])
            nc.gpsimd.collective_compute(
                "AllGather", mybir.AluOpType.bypass,
                ins=[NSL[:]], outs=[NFL[:]],
                replica_groups=[list(range(CORES))])
            nc.sync.dma_start(out=OUT[:].rearrange("(a p) c -> p a c", p=128),
                              in_=t[:, :100 * F].rearrange("p (a c) -> p a c", c=F))
    nc.finalize()
    return nc


def kernel(**inputs):
    global LAST_RESULTS, LAST_NC, LAST_IN_MAPS, LAST_PLANS
    in_maps, plans, unmap = _prepare(inputs)
    nc = _build(plans)
    LAST_NC, LAST_IN_MAPS, LAST_PLANS = nc, in_maps, plans
    trace = bool(int(os.environ.get("KERNEL_TRACE", "0")))
    res = run_bass_kernel_spmd(nc, in_maps, list(range(CORES)), trace=trace)
    LAST_RESULTS = res

    node_grid, valid = unmap["node_grid"], unmap["valid"]
    out = np.zeros((N_NODES, NCLUST), np.float32)
    for k in range(CORES):
        rk = np.asarray(res.results[k]["out"], np.float32)
        for q in range(4):
            u = q * CORES + k
            v = valid[u]
            out[node_grid[u, v]] = rk[q * J_UNIT:(q + 1) * J_UNIT][v][:, :NCLUST]
    return out

